# revision 14
# baseline (speedup 1.0000x reference)
# Trainium2 Bass kernel for nn_CompCSD (segment_reduce):
#   vmf = softmax(vmf_activations, axis=K)
#   content[b,l,h,w]  = sum_{k: label[k]==l} vmf[b,k,h,w]
#   features[b,c,h,w] = sum_k vmf[b,k,h,w] * content[b,label[k],h,w] * kernels[k,c]
#
# Sharding: 8 cores, data-parallel over (batch, H-half): core i -> b=i//2,
# h0=(i%2)*64.  Per core: pixels = 64*128 = 8192, K=256, C=64, L=8.
#
# Device layout per core ("layout B"): K on partitions (2 tiles of 128),
# pixels on the free axis, processed in 16 chunks of 512 pixels.
# Per chunk:
#   e = exp(a)                                  (ACT, one op on [128,2,512])
#   cu9T[pix,j,l] = sum_k e[k,pix] * oh9[k,l]   (PE, 8 small matmuls -> PSUM;
#       col 8 of oh9 is all-ones so cu9T[:,:,8] is the softmax denominator D
#       (col 9 is zero padding: fp32r ISA needs even innermost free extents),
#       cols 0..7 are the per-segment sums, all already transposed to
#       pixel-on-partition layout so the per-pixel normalizers are cheap)
#   invdT = 1/D ; i2T = invdT^2                 (DVE, tiny [128,4] ops)
#   contentT = cu9T[:,:,0:8] * invdT            (DVE, -> SBUF accumulator)
#   cnT      = cu9T[:,:,0:8] * i2T              (DVE)
#   cn       = transpose(cnT) -> [8, 512]       (PE transpose via identity)
#   cg[k,pix] = sum_l sel[l,k] * cn[l,pix]      (PE, gathers cn[label[k]])
#   scaled = e * cg                             (DVE, [128,2,512])
#   fu[c,pix] = sum_k kern[k,c] * scaled[k,pix] (PE -> PSUM, already normalized)
#   feat out via ACT copy PSUM->SBUF + DMA
# content is written once at the end in transposed layout and fixed on host.
#
# Matmul inputs are viewed as float32r (single-pass fp32 on the PE array,
# 1 cycle/row at N>=256 vs 4 for plain fp32).

import numpy as np

B, K, H, W, C = 4, 256, 128, 128, 64
L = 8
NCORES = 8
PIX = H * W // 2        # 8192 pixels per core
NPIX = 512              # pixels per chunk
NCHUNK = PIX // NPIX    # 16
KT = 2                  # K tiles of 128
USE_F32R = True

_prog_cache = {}


def _build_program(use_f32r=USE_F32R, rep=1):
    import concourse.bass as bass
    import concourse.mybir as mybir
    import concourse.tile as tile
    from concourse import bacc
    from concourse.masks import make_identity

    f32 = mybir.dt.float32
    nc = bacc.Bacc("TRN2", target_bir_lowering=False)

    vmf = nc.dram_tensor("vmf", [K, PIX], f32, kind="ExternalInput")
    oh9 = nc.dram_tensor("oh9", [128, KT, L + 2], f32, kind="ExternalInput")
    sel = nc.dram_tensor("sel", [L, KT, 128], f32, kind="ExternalInput")
    kern = nc.dram_tensor("kern", [128, KT, C], f32, kind="ExternalInput")
    feat = nc.dram_tensor("feat", [C, PIX], f32, kind="ExternalOutput")
    contT = nc.dram_tensor("contT", [128, NCHUNK * 4, L], f32, kind="ExternalOutput")

    def r(ap):
        # f32r view: used on matmul inputs AND on the producing instruction's
        # output (walrus checkMatmultFP32r requires producers of f32r matmul
        # inputs to emit float32r, i.e. "rounded").
        return ap.bitcast(mybir.dt.float32r) if use_f32r else ap

    with tile.TileContext(nc) as tc:
        with (
            tc.tile_pool(name="consts", bufs=1) as consts,
            tc.tile_pool(name="io", bufs=3) as io,
            tc.tile_pool(name="work", bufs=2) as work,
            tc.tile_pool(name="accp", bufs=1) as accp,
            tc.tile_pool(name="ps_small", bufs=2, space="PSUM") as ps_small,
            tc.tile_pool(name="ps_big", bufs=1, space="PSUM") as ps_big,
            tc.tile_pool(name="ps_fu", bufs=2, space="PSUM") as ps_fu,
        ):
            sb_oh9 = consts.tile([128, KT, L + 2], f32)
            nc.sync.dma_start(out=r(sb_oh9), in_=r(oh9[:, :, :]))
            sb_sel = consts.tile([L, KT, 128], f32)
            nc.sync.dma_start(out=r(sb_sel), in_=r(sel[:, :, :]))
            sb_kern = consts.tile([128, KT, C], f32)
            nc.sync.dma_start(out=r(sb_kern), in_=r(kern[:, :, :]))
            ident = consts.tile([128, 128], f32)
            make_identity(nc, ident)

            contT_acc = accp.tile([128, NCHUNK * 4, L], f32)

            vmf_r = vmf[:, :].rearrange("(t p) x -> p t x", t=KT)

            # Input is streamed in groups of GRP chunks: one DMA + one exp per
            # group (bigger descriptors, fewer instruction overheads).
            GRP = 4
            GPIX = GRP * NPIX
            grp_tiles = {}

            for c in [ci for _ in range(rep) for ci in range(NCHUNK)]:
                xs = bass.ds(c * NPIX, NPIX)

                g, sub = c // GRP, c % GRP
                if sub == 0:
                    e_in = io.tile([128, KT, GPIX], f32)
                    nc.sync.dma_start(
                        out=e_in, in_=vmf_r[:, :, bass.ds(g * GPIX, GPIX)]
                    )
                    e_g = work.tile([128, KT, GPIX], f32, tag="e_g")
                    nc.scalar.activation(
                        out=r(e_g), in_=e_in, func=mybir.ActivationFunctionType.Exp
                    )
                    grp_tiles[g] = e_g
                e = grp_tiles[g][:, :, bass.ds(sub * NPIX, NPIX)]

                cu9T = ps_small.tile([128, 4, L + 2], f32)
                for j in range(4):
                    for t in range(KT):
                        nc.tensor.matmul(
                            cu9T[:, j, :],
                            r(e[:, t, bass.ds(j * 128, 128)]),
                            r(sb_oh9[:, t, :]),
                            start=(t == 0),
                            stop=(t == KT - 1),
                        )

                invdT = work.tile([128, 4], f32)
                nc.vector.reciprocal(out=invdT, in_=cu9T[:, :, L])
                i2T = work.tile([128, 4], f32)
                nc.vector.tensor_mul(i2T, invdT, invdT)

                nc.vector.tensor_mul(
                    contT_acc[:, c * 4 : (c + 1) * 4, :],
                    cu9T[:, :, 0:L],
                    invdT[:, :, None].broadcast_to([128, 4, L]),
                )
                cnT = work.tile([128, 4, L], f32)
                nc.vector.tensor_mul(
                    cnT,
                    cu9T[:, :, 0:L],
                    i2T[:, :, None].broadcast_to([128, 4, L]),
                )

                cn_ps = ps_small.tile([L, 4, 128], f32)
                for j in range(4):
                    nc.tensor.transpose(cn_ps[:, j, :], cnT[:, j, :], ident)
                cn_sb = work.tile([L, 4, 128], f32)
                nc.scalar.copy(out=r(cn_sb), in_=cn_ps)

                cg = ps_big.tile([128, KT, NPIX], f32)
                for t in range(KT):
                    nc.tensor.matmul(
                        cg[:, t, :],
                        r(sb_sel[:, t, :]),
                        r(cn_sb[:, :, :]),
                        start=True,
                        stop=True,
                    )

                scaled = work.tile([128, KT, NPIX], f32)
                nc.vector.tensor_mul(r(scaled), e, cg)

                fu = ps_fu.tile([C, NPIX], f32)
                for t in range(KT):
                    nc.tensor.matmul(
                        fu,
                        r(sb_kern[:, t, :]),
                        r(scaled[:, t, :]),
                        start=(t == 0),
                        stop=(t == KT - 1),
                    )
                fu_sb = io.tile([C, NPIX], f32)
                # split the PSUM->SBUF copy across ACT and DVE to balance load
                nc.scalar.copy(out=fu_sb[:, 0:320], in_=fu[:, 0:320])
                nc.vector.tensor_copy(fu_sb[:, 320:NPIX], fu[:, 320:NPIX])
                nc.gpsimd.dma_start(out=feat[:, xs], in_=fu_sb)

            nc.gpsimd.dma_start(out=contT[:, :, :], in_=contT_acc)

    nc.finalize()
    return nc


def _get_program(rep=1):
    key = ("prog", USE_F32R, rep)
    if key not in _prog_cache:
        _prog_cache[key] = _build_program(rep=rep)
    return _prog_cache[key]


def _make_consts(kernels, labels):
    oh9 = np.zeros((128, KT, L + 2), np.float32)
    sel = np.zeros((L, KT, 128), np.float32)
    kern = np.zeros((128, KT, C), np.float32)
    ar = np.arange(128)
    for t in range(KT):
        lab_t = labels[t * 128 : (t + 1) * 128]
        oh9[ar, t, lab_t] = 1.0
        oh9[:, t, L] = 1.0
        sel[lab_t, t, ar] = 1.0
        kern[:, t, :] = kernels[t * 128 : (t + 1) * 128, :]
    return oh9, sel, kern


def _run(inputs, trace=False):
    from concourse.bass_utils import run_bass_kernel_spmd

    vmf = np.ascontiguousarray(np.asarray(inputs["vmf_activations"], dtype=np.float32))
    kernels = np.asarray(inputs["kernels"], dtype=np.float32)
    labels = np.asarray(inputs["kernel_labels"]).astype(np.int64)

    oh9, sel, kern = _make_consts(kernels, labels)

    in_maps = []
    for i in range(NCORES):
        b, h0 = i // 2, (i % 2) * 64
        shard = np.ascontiguousarray(vmf[b, :, h0 : h0 + 64, :].reshape(K, PIX))
        in_maps.append({"vmf": shard, "oh9": oh9, "sel": sel, "kern": kern})

    nc = _get_program()
    res = run_bass_kernel_spmd(nc, in_maps, core_ids=list(range(NCORES)), trace=trace)

    content = np.zeros((B, L, H, W), np.float32)
    features = np.zeros((B, C, H, W), np.float32)
    for i, rd in enumerate(res.results):
        b, h0 = i // 2, (i % 2) * 64
        features[b, :, h0 : h0 + 64, :] = rd["feat"].reshape(C, 64, W)
        ct = rd["contT"].reshape(128, NCHUNK, 4, L)
        content[b, :, h0 : h0 + 64, :] = ct.transpose(3, 1, 2, 0).reshape(L, 64, W)
    return (content, features), res


def kernel(**inputs):
    out, _ = _run(inputs, trace=False)
    return out


def _make_in_maps(inputs):
    vmf = np.ascontiguousarray(np.asarray(inputs["vmf_activations"], dtype=np.float32))
    kernels = np.asarray(inputs["kernels"], dtype=np.float32)
    labels = np.asarray(inputs["kernel_labels"]).astype(np.int64)
    oh9, sel, kern = _make_consts(kernels, labels)
    in_maps = []
    for i in range(NCORES):
        b, h0 = i // 2, (i % 2) * 64
        shard = np.ascontiguousarray(vmf[b, :, h0 : h0 + 64, :].reshape(K, PIX))
        in_maps.append({"vmf": shard, "oh9": oh9, "sel": sel, "kern": kern})
    return in_maps


def _make_timing_fn(nc, in_maps):
    """Build a non-donating jitted runner for nc; returns (fn, dev_args)."""
    import jax
    from jax.sharding import Mesh, PartitionSpec
    from jax.experimental.shard_map import shard_map
    import concourse.mybir as mybir
    from concourse import bass2jax

    bass2jax.install_neuronx_cc_hook()

    partition_name = nc.partition_id_tensor.name if nc.partition_id_tensor else None
    in_names, out_names, out_avals, zero_outs = [], [], [], []
    for alloc in nc.m.functions[0].allocations:
        if not isinstance(alloc, mybir.MemoryLocationSet):
            continue
        name = alloc.memorylocations[0].name
        if alloc.kind == "ExternalInput":
            if name != partition_name:
                in_names.append(name)
        elif alloc.kind == "ExternalOutput":
            shape = tuple(alloc.tensor_shape)
            dtype = mybir.dt.np(alloc.dtype)
            out_names.append(name)
            out_avals.append(jax.core.ShapedArray(shape, dtype))
            zero_outs.append(np.zeros(shape, dtype))
    n_params = len(in_names)
    all_in_names = in_names + out_names
    if partition_name is not None:
        all_in_names = all_in_names + [partition_name]

    def _body(*args):
        operands = list(args)
        if partition_name is not None:
            operands.append(bass2jax.partition_id_tensor())
        outs = bass2jax._bass_exec_p.bind(
            *operands,
            out_avals=tuple(out_avals),
            in_names=tuple(all_in_names),
            out_names=tuple(out_names),
            lowering_input_output_aliases=(),
            sim_require_finite=True,
            sim_require_nnan=True,
            nc=nc,
        )
        return tuple(outs)

    devices = jax.devices()[:NCORES]
    mesh = Mesh(np.asarray(devices), ("core",))
    n_outs = len(out_names)
    in_specs = (PartitionSpec("core"),) * (n_params + n_outs)
    out_specs = (PartitionSpec("core"),) * n_outs
    fn = jax.jit(
        shard_map(_body, mesh=mesh, in_specs=in_specs, out_specs=out_specs,
                  check_rep=False),
        keep_unused=True,
    )
    concat_in = [
        np.concatenate([np.asarray(m[nm]) for m in in_maps], axis=0)
        for nm in in_names
    ]
    concat_zero = [
        np.zeros((NCORES * z.shape[0], *z.shape[1:]), z.dtype) for z in zero_outs
    ]
    args = [jax.device_put(a) for a in concat_in + concat_zero]
    return fn, args


def _time_fn(fn, args, iters, warmup=3):
    import jax
    import time as _time

    for _ in range(warmup):
        outs = fn(*args)
    jax.block_until_ready(outs)
    best = float("inf")
    for _ in range(3):
        t0 = _time.perf_counter()
        for _ in range(iters):
            outs = fn(*args)
        jax.block_until_ready(outs)
        best = min(best, (_time.perf_counter() - t0) / iters)
    return best * 1e9


def time_hw(inputs, iters=30, repn=5):
    """Device time per workload pass, measured as the marginal wall-clock cost
    of extra on-device repetitions: (T(repn) - T(1)) / (repn - 1).  All fixed
    per-dispatch overheads (axon round trip, DGE setup, input DMA from host)
    cancel in the difference."""
    in_maps = _make_in_maps(inputs)
    fn1, args1 = _make_timing_fn(_get_program(rep=1), in_maps)
    fnN, argsN = _make_timing_fn(_get_program(rep=repn), in_maps)
    t1 = _time_fn(fn1, args1, iters)
    tN = _time_fn(fnN, argsN, iters)
    print(f"  [time_hw] T(rep=1)={t1:.0f} ns, T(rep={repn})={tN:.0f} ns")
    return (tN - t1) / (repn - 1)


# revision 17
# speedup vs baseline: 1.3018x; 1.3018x over previous
# Trainium2 Bass kernel for nn_CompCSD (segment_reduce):
#   vmf = softmax(vmf_activations, axis=K)
#   content[b,l,h,w]  = sum_{k: label[k]==l} vmf[b,k,h,w]
#   features[b,c,h,w] = sum_k vmf[b,k,h,w] * content[b,label[k],h,w] * kernels[k,c]
#
# Sharding: 8 cores, data-parallel over (batch, H-half): core i -> b=i//2,
# h0=(i%2)*64.  Per core: pixels = 64*128 = 8192, K=256, C=64, L=8.
#
# Device layout per core: K on partitions (2 tiles of 128), pixels on the
# free axis, 16 chunks of 512 pixels, input DMA + exp at 2048-pixel group
# granularity.  Per chunk ("v5 classic" — exactly 6 PE matmuls, measured PE
# floor is ~213ns/matmul so instruction count dominates over FLOPs):
#   e = exp(a)                                   ACT  [128,2,2048] per group
#   cu9[l,pix] = sum_k oh9[k,l] e[k,pix]         PE   2 mm -> PSUM [10,512]
#       (col 8 of oh9 = 1 -> row 8 = softmax denominator D; col 9 = pad:
#        fp32r ISA needs even innermost extents)
#   cu9_sb = copy(cu9)                           ACT  PSUM->SBUF
#   invd = 1/D                                   DVE  [1,512] row
#   i2 = invd^2                                  DVE  [1,512]
#   i2b8 = partition_broadcast(i2) to 8 rows     Pool
#   cn = cu9_sb[0:8] * i2b8                      Pool[0:304] + DVE[304:512]
#   cg[k,pix] = sum_l sel[l,k] cn[l,pix]         PE   2 mm (content gather)
#   scaled = e * cg                              DVE  [128,2,512]
#   fu[c,pix] = sum_k kern[k,c] scaled[k,pix]    PE   2 mm (normalized: cn
#                                                     carries invd^2)
#   fu_sb = copy(fu)                             ACT  PSUM->SBUF -> DMA
#   cu8 rows (unnormalized content) and the invd row are DMA'd out raw;
#   the host multiplies content = cu8 * invd (tiny), avoiding one more
#   per-chunk [8,512] elementwise op on device.
# All matmul inputs are float32r (single-pass fp32, 1 cyc/row at N>=256).

import numpy as np

B, K, H, W, C = 4, 256, 128, 128, 64
L = 8
NCORES = 8
PIX = H * W // 2        # 8192 pixels per core
NPIX = 512              # pixels per chunk
NCHUNK = PIX // NPIX    # 16
KT = 2                  # K tiles of 128
USE_F32R = True
CNSPLIT = 304           # cn columns done on Pool; rest on DVE

_prog_cache = {}


def _build_program(use_f32r=USE_F32R, rep=1):
    import concourse.bass as bass
    import concourse.mybir as mybir
    import concourse.tile as tile
    from concourse import bacc

    f32 = mybir.dt.float32
    nc = bacc.Bacc("TRN2", target_bir_lowering=False)

    vmf = nc.dram_tensor("vmf", [K, PIX], f32, kind="ExternalInput")
    oh9 = nc.dram_tensor("oh9", [128, KT, L + 2], f32, kind="ExternalInput")
    sel = nc.dram_tensor("sel", [L + 2, KT, 128], f32, kind="ExternalInput")
    kern = nc.dram_tensor("kern", [128, KT, C], f32, kind="ExternalInput")
    feat = nc.dram_tensor("feat", [C, PIX], f32, kind="ExternalOutput")
    cu8o = nc.dram_tensor("cu8o", [L, PIX], f32, kind="ExternalOutput")
    invdo = nc.dram_tensor("invdo", [1, PIX], f32, kind="ExternalOutput")

    def r(ap):
        # f32r view: on matmul inputs AND the producing instruction's output
        # (walrus checkMatmultFP32r requires producers to emit float32r).
        return ap.bitcast(mybir.dt.float32r) if use_f32r else ap

    with tile.TileContext(nc) as tc:
        with (
            tc.tile_pool(name="consts", bufs=1) as consts,
            tc.tile_pool(name="io", bufs=2) as io,
            tc.tile_pool(name="work", bufs=3) as work,
            tc.tile_pool(name="accp", bufs=1) as accp,
            tc.tile_pool(name="ps_cu", bufs=2, space="PSUM") as ps_cu,
            tc.tile_pool(name="ps_cg", bufs=1, space="PSUM") as ps_cg,
            tc.tile_pool(name="ps_fu", bufs=2, space="PSUM") as ps_fu,
        ):
            sb_oh9 = consts.tile([128, KT, L + 2], f32)
            nc.sync.dma_start(out=r(sb_oh9), in_=r(oh9[:, :, :]))
            sb_sel = consts.tile([L + 2, KT, 128], f32)
            nc.sync.dma_start(out=r(sb_sel), in_=r(sel[:, :, :]))
            sb_kern = consts.tile([128, KT, C], f32)
            nc.sync.dma_start(out=r(sb_kern), in_=r(kern[:, :, :]))

            invd_acc = accp.tile([1, PIX], f32)

            vmf_r = vmf[:, :].rearrange("(t p) x -> p t x", t=KT)

            GRP = 4
            GPIX = GRP * NPIX
            grp_tiles = {}

            for c in [ci for _ in range(rep) for ci in range(NCHUNK)]:
                xs = bass.ds(c * NPIX, NPIX)

                g, sub = c // GRP, c % GRP
                if sub == 0:
                    e_in = io.tile([128, KT, GPIX], f32)
                    nc.sync.dma_start(
                        out=e_in, in_=vmf_r[:, :, bass.ds(g * GPIX, GPIX)]
                    )
                    e_g = work.tile([128, KT, GPIX], f32, tag="e_g", bufs=2)
                    nc.scalar.activation(
                        out=r(e_g), in_=e_in, func=mybir.ActivationFunctionType.Exp
                    )
                    grp_tiles[g] = e_g
                e = grp_tiles[g][:, :, bass.ds(sub * NPIX, NPIX)]

                # cu9[l, pix] = sum_k oh9[k, l] * e[k, pix]  (rows 0..7 =
                # per-segment sums, row 8 = denominator, row 9 = pad)
                cu9 = ps_cu.tile([L + 2, NPIX], f32)
                for t in range(KT):
                    nc.tensor.matmul(
                        cu9,
                        r(sb_oh9[:, t, :]),
                        r(e[:, t, :]),
                        start=(t == 0),
                        stop=(t == KT - 1),
                    )
                cu9_sb = work.tile([L + 2, NPIX], f32)
                nc.scalar.copy(out=cu9_sb, in_=cu9)
                nc.sync.dma_start(out=cu8o[:, xs], in_=cu9_sb[1 : L + 1, :])

                # row 0 of cu9 is the denominator (oh9 col 0 = ones); rows
                # 1..8 are the segment sums.  Compute engines must start
                # partition access at 0, so all row ops span rows 0..9 and
                # the gather matmul contracts over all 10 rows with zero
                # weights on rows 0 and 9.
                invd = invd_acc[0:1, xs]
                nc.vector.reciprocal(out=invd, in_=cu9_sb[0:1, :])
                i2 = work.tile([1, NPIX], f32)
                nc.vector.tensor_mul(i2, invd, invd)
                i2b = work.tile([L + 2, NPIX], f32)
                nc.gpsimd.partition_broadcast(i2b, i2)

                cn = work.tile([L + 2, NPIX], f32)
                nc.gpsimd.tensor_mul(
                    r(cn[:, 0:CNSPLIT]), cu9_sb[:, 0:CNSPLIT], i2b[:, 0:CNSPLIT]
                )
                nc.vector.tensor_mul(
                    r(cn[:, CNSPLIT:NPIX]),
                    cu9_sb[:, CNSPLIT:NPIX],
                    i2b[:, CNSPLIT:NPIX],
                )

                cg = ps_cg.tile([128, KT, NPIX], f32)
                for t in range(KT):
                    nc.tensor.matmul(
                        cg[:, t, :],
                        r(sb_sel[:, t, :]),
                        r(cn[:, :]),
                        start=True,
                        stop=True,
                    )

                scaled = work.tile([128, KT, NPIX], f32)
                nc.vector.tensor_mul(r(scaled), e, cg)

                fu = ps_fu.tile([C, NPIX], f32)
                for t in range(KT):
                    nc.tensor.matmul(
                        fu,
                        r(sb_kern[:, t, :]),
                        r(scaled[:, t, :]),
                        start=(t == 0),
                        stop=(t == KT - 1),
                    )
                fu_sb = io.tile([C, NPIX], f32, bufs=3)
                nc.scalar.copy(out=fu_sb, in_=fu)
                nc.sync.dma_start(out=feat[:, xs], in_=fu_sb)

            nc.sync.dma_start(out=invdo[:, :], in_=invd_acc)

    nc.finalize()
    return nc


def _get_program(rep=1):
    key = ("prog", USE_F32R, rep)
    if key not in _prog_cache:
        _prog_cache[key] = _build_program(rep=rep)
    return _prog_cache[key]


def _make_consts(kernels, labels):
    # oh9 col 0 = ones (denominator), cols 1..8 = one-hot segment, col 9 = 0.
    # sel row 1+l selects segment l; rows 0 and 9 are zero.
    oh9 = np.zeros((128, KT, L + 2), np.float32)
    sel = np.zeros((L + 2, KT, 128), np.float32)
    kern = np.zeros((128, KT, C), np.float32)
    ar = np.arange(128)
    for t in range(KT):
        lab_t = labels[t * 128 : (t + 1) * 128]
        oh9[:, t, 0] = 1.0
        oh9[ar, t, 1 + lab_t] = 1.0
        sel[1 + lab_t, t, ar] = 1.0
        kern[:, t, :] = kernels[t * 128 : (t + 1) * 128, :]
    return oh9, sel, kern


def _make_in_maps(inputs):
    vmf = np.ascontiguousarray(np.asarray(inputs["vmf_activations"], dtype=np.float32))
    kernels = np.asarray(inputs["kernels"], dtype=np.float32)
    labels = np.asarray(inputs["kernel_labels"]).astype(np.int64)
    oh9, sel, kern = _make_consts(kernels, labels)
    in_maps = []
    for i in range(NCORES):
        b, h0 = i // 2, (i % 2) * 64
        shard = np.ascontiguousarray(vmf[b, :, h0 : h0 + 64, :].reshape(K, PIX))
        in_maps.append({"vmf": shard, "oh9": oh9, "sel": sel, "kern": kern})
    return in_maps


def _run(inputs, trace=False):
    from concourse.bass_utils import run_bass_kernel_spmd

    in_maps = _make_in_maps(inputs)
    nc = _get_program()
    res = run_bass_kernel_spmd(nc, in_maps, core_ids=list(range(NCORES)), trace=trace)

    content = np.zeros((B, L, H, W), np.float32)
    features = np.zeros((B, C, H, W), np.float32)
    for i, rd in enumerate(res.results):
        b, h0 = i // 2, (i % 2) * 64
        features[b, :, h0 : h0 + 64, :] = rd["feat"].reshape(C, 64, W)
        cont = rd["cu8o"] * rd["invdo"][0][None, :]
        content[b, :, h0 : h0 + 64, :] = cont.reshape(L, 64, W)
    return (content, features), res


def kernel(**inputs):
    out, _ = _run(inputs, trace=False)
    return out


def _make_timing_fn(nc, in_maps):
    """Build a non-donating jitted runner for nc; returns (fn, dev_args)."""
    import jax
    from jax.sharding import Mesh, PartitionSpec
    from jax.experimental.shard_map import shard_map
    import concourse.mybir as mybir
    from concourse import bass2jax

    bass2jax.install_neuronx_cc_hook()

    partition_name = nc.partition_id_tensor.name if nc.partition_id_tensor else None
    in_names, out_names, out_avals, zero_outs = [], [], [], []
    for alloc in nc.m.functions[0].allocations:
        if not isinstance(alloc, mybir.MemoryLocationSet):
            continue
        name = alloc.memorylocations[0].name
        if alloc.kind == "ExternalInput":
            if name != partition_name:
                in_names.append(name)
        elif alloc.kind == "ExternalOutput":
            shape = tuple(alloc.tensor_shape)
            dtype = mybir.dt.np(alloc.dtype)
            out_names.append(name)
            out_avals.append(jax.core.ShapedArray(shape, dtype))
            zero_outs.append(np.zeros(shape, dtype))
    n_params = len(in_names)
    all_in_names = in_names + out_names
    if partition_name is not None:
        all_in_names = all_in_names + [partition_name]

    def _body(*args):
        operands = list(args)
        if partition_name is not None:
            operands.append(bass2jax.partition_id_tensor())
        outs = bass2jax._bass_exec_p.bind(
            *operands,
            out_avals=tuple(out_avals),
            in_names=tuple(all_in_names),
            out_names=tuple(out_names),
            lowering_input_output_aliases=(),
            sim_require_finite=True,
            sim_require_nnan=True,
            nc=nc,
        )
        return tuple(outs)

    devices = jax.devices()[:NCORES]
    mesh = Mesh(np.asarray(devices), ("core",))
    n_outs = len(out_names)
    in_specs = (PartitionSpec("core"),) * (n_params + n_outs)
    out_specs = (PartitionSpec("core"),) * n_outs
    fn = jax.jit(
        shard_map(_body, mesh=mesh, in_specs=in_specs, out_specs=out_specs,
                  check_rep=False),
        keep_unused=True,
    )
    concat_in = [
        np.concatenate([np.asarray(m[nm]) for m in in_maps], axis=0)
        for nm in in_names
    ]
    concat_zero = [
        np.zeros((NCORES * z.shape[0], *z.shape[1:]), z.dtype) for z in zero_outs
    ]
    args = [jax.device_put(a) for a in concat_in + concat_zero]
    return fn, args


def time_hw_ntff(inputs, n=3):
    """True HW exec time via NTFF profiling (axon hook shim): runs the kernel
    n times with trace=True and reports the best max-core exec_time_ns."""
    import ntff_hook

    ntff_hook.install()
    import os

    os.environ["BASS_PERFETTO_PROFILE_ALL_CORES"] = "1"
    best = None
    for _ in range(n):
        _, res = _run(inputs, trace=True)
        if res.exec_time_ns is not None:
            best = res.exec_time_ns if best is None else min(best, res.exec_time_ns)
    return best


# revision 18
# speedup vs baseline: 3.5891x; 2.7571x over previous
# Trainium2 Bass kernel for nn_CompCSD (segment_reduce):
#   vmf = softmax(vmf_activations, axis=K)
#   content[b,l,h,w]  = sum_{k: label[k]==l} vmf[b,k,h,w]
#   features[b,c,h,w] = sum_k vmf[b,k,h,w] * content[b,label[k],h,w] * kernels[k,c]
#
# Sharding: 8 cores, data-parallel over (batch, H-half): core i -> b=i//2,
# h0=(i%2)*64.  Per core: pixels = 64*128 = 8192, K=256, C=64, L=8.
#
# Device layout per core ("layout B"): K on partitions (2 tiles of 128),
# pixels on the free axis, processed in 16 chunks of 512 pixels.
# Per chunk:
#   e = exp(a)                                  (ACT, one op on [128,2,512])
#   cu9T[pix,j,l] = sum_k e[k,pix] * oh9[k,l]   (PE, 8 small matmuls -> PSUM;
#       col 8 of oh9 is all-ones so cu9T[:,:,8] is the softmax denominator D
#       (col 9 is zero padding: fp32r ISA needs even innermost free extents),
#       cols 0..7 are the per-segment sums, all already transposed to
#       pixel-on-partition layout so the per-pixel normalizers are cheap)
#   invdT = 1/D ; i2T = invdT^2                 (DVE, tiny [128,4] ops)
#   contentT = cu9T[:,:,0:8] * invdT            (DVE, -> SBUF accumulator)
#   cnT      = cu9T[:,:,0:8] * i2T              (DVE)
#   cn       = transpose(cnT) -> [8, 512]       (PE transpose via identity)
#   cg[k,pix] = sum_l sel[l,k] * cn[l,pix]      (PE, gathers cn[label[k]])
#   scaled = e * cg                             (DVE, [128,2,512])
#   fu[c,pix] = sum_k kern[k,c] * scaled[k,pix] (PE -> PSUM, already normalized)
#   feat out via ACT copy PSUM->SBUF + DMA
# content is written once at the end in transposed layout and fixed on host.
#
# Matmul inputs are viewed as float32r (single-pass fp32 on the PE array,
# 1 cycle/row at N>=256 vs 4 for plain fp32).

import numpy as np

B, K, H, W, C = 4, 256, 128, 128, 64
L = 8
NCORES = 8
PIX = H * W // 2        # 8192 pixels per core
NPIX = 512              # pixels per chunk
NCHUNK = PIX // NPIX    # 16
KT = 2                  # K tiles of 128
USE_F32R = True

_prog_cache = {}


def _build_program(use_f32r=USE_F32R, rep=1):
    import concourse.bass as bass
    import concourse.mybir as mybir
    import concourse.tile as tile
    from concourse import bacc
    from concourse.masks import make_identity

    f32 = mybir.dt.float32
    nc = bacc.Bacc("TRN2", target_bir_lowering=False)

    vmf = nc.dram_tensor("vmf", [K, PIX], f32, kind="ExternalInput")
    oh9 = nc.dram_tensor("oh9", [128, KT, L + 2], f32, kind="ExternalInput")
    sel = nc.dram_tensor("sel", [L, KT, 128], f32, kind="ExternalInput")
    kern = nc.dram_tensor("kern", [128, KT, C], f32, kind="ExternalInput")
    feat = nc.dram_tensor("feat", [C, PIX], f32, kind="ExternalOutput")
    contT = nc.dram_tensor("contT", [128, NCHUNK * 4, L], f32, kind="ExternalOutput")

    def r(ap):
        # f32r view: used on matmul inputs AND on the producing instruction's
        # output (walrus checkMatmultFP32r requires producers of f32r matmul
        # inputs to emit float32r, i.e. "rounded").
        return ap.bitcast(mybir.dt.float32r) if use_f32r else ap

    with tile.TileContext(nc) as tc:
        with (
            tc.tile_pool(name="consts", bufs=1) as consts,
            tc.tile_pool(name="io", bufs=3) as io,
            tc.tile_pool(name="work", bufs=2) as work,
            tc.tile_pool(name="accp", bufs=1) as accp,
            tc.tile_pool(name="ps_small", bufs=2, space="PSUM") as ps_small,
            tc.tile_pool(name="ps_big", bufs=1, space="PSUM") as ps_big,
            tc.tile_pool(name="ps_fu", bufs=2, space="PSUM") as ps_fu,
        ):
            sb_oh9 = consts.tile([128, KT, L + 2], f32)
            nc.sync.dma_start(out=r(sb_oh9), in_=r(oh9[:, :, :]))
            sb_sel = consts.tile([L, KT, 128], f32)
            nc.sync.dma_start(out=r(sb_sel), in_=r(sel[:, :, :]))
            sb_kern = consts.tile([128, KT, C], f32)
            nc.sync.dma_start(out=r(sb_kern), in_=r(kern[:, :, :]))
            ident = consts.tile([128, 128], f32)
            make_identity(nc, ident)

            contT_acc = accp.tile([128, NCHUNK * 4, L], f32)

            vmf_r = vmf[:, :].rearrange("(t p) x -> p t x", t=KT)

            # Input is streamed in groups of GRP chunks: one DMA + one exp per
            # group (bigger descriptors, fewer instruction overheads).
            GRP = 4
            GPIX = GRP * NPIX
            grp_tiles = {}

            for c in [ci for _ in range(rep) for ci in range(NCHUNK)]:
                xs = bass.ds(c * NPIX, NPIX)

                g, sub = c // GRP, c % GRP
                if sub == 0:
                    e_in = io.tile([128, KT, GPIX], f32)
                    nc.sync.dma_start(
                        out=e_in, in_=vmf_r[:, :, bass.ds(g * GPIX, GPIX)]
                    )
                    e_g = work.tile([128, KT, GPIX], f32, tag="e_g")
                    nc.scalar.activation(
                        out=r(e_g), in_=e_in, func=mybir.ActivationFunctionType.Exp
                    )
                    grp_tiles[g] = e_g
                e = grp_tiles[g][:, :, bass.ds(sub * NPIX, NPIX)]

                cu9T = ps_small.tile([128, 4, L + 2], f32)
                for j in range(4):
                    for t in range(KT):
                        nc.tensor.matmul(
                            cu9T[:, j, :],
                            r(e[:, t, bass.ds(j * 128, 128)]),
                            r(sb_oh9[:, t, :]),
                            start=(t == 0),
                            stop=(t == KT - 1),
                        )

                invdT = work.tile([128, 4], f32)
                nc.vector.reciprocal_approx_fast(out=invdT, in_=cu9T[:, :, L])
                i2T = work.tile([128, 4], f32)
                nc.vector.tensor_mul(i2T, invdT, invdT)

                nc.vector.tensor_mul(
                    contT_acc[:, c * 4 : (c + 1) * 4, :],
                    cu9T[:, :, 0:L],
                    invdT[:, :, None].broadcast_to([128, 4, L]),
                )
                cnT = work.tile([128, 4, L], f32)
                nc.vector.tensor_mul(
                    cnT,
                    cu9T[:, :, 0:L],
                    i2T[:, :, None].broadcast_to([128, 4, L]),
                )

                cn_ps = ps_small.tile([L, 4, 128], f32)
                for j in range(4):
                    nc.tensor.transpose(cn_ps[:, j, :], cnT[:, j, :], ident)
                cn_sb = work.tile([L, 4, 128], f32)
                nc.scalar.copy(out=r(cn_sb), in_=cn_ps)

                cg = ps_big.tile([128, KT, NPIX], f32)
                for t in range(KT):
                    nc.tensor.matmul(
                        cg[:, t, :],
                        r(sb_sel[:, t, :]),
                        r(cn_sb[:, :, :]),
                        start=True,
                        stop=True,
                    )

                scaled = work.tile([128, KT, NPIX], f32)
                nc.vector.tensor_mul(r(scaled), e, cg)

                fu = ps_fu.tile([C, NPIX], f32)
                for t in range(KT):
                    nc.tensor.matmul(
                        fu,
                        r(sb_kern[:, t, :]),
                        r(scaled[:, t, :]),
                        start=(t == 0),
                        stop=(t == KT - 1),
                    )
                fu_sb = io.tile([C, NPIX], f32)
                # split the PSUM->SBUF copy across ACT and DVE to balance load
                nc.scalar.copy(out=fu_sb[:, 0:320], in_=fu[:, 0:320])
                nc.vector.tensor_copy(fu_sb[:, 320:NPIX], fu[:, 320:NPIX])
                nc.gpsimd.dma_start(out=feat[:, xs], in_=fu_sb)

            nc.gpsimd.dma_start(out=contT[:, :, :], in_=contT_acc)

    nc.finalize()
    return nc


def _get_program(rep=1):
    key = ("prog", USE_F32R, rep)
    if key not in _prog_cache:
        _prog_cache[key] = _build_program(rep=rep)
    return _prog_cache[key]


def _make_consts(kernels, labels):
    oh9 = np.zeros((128, KT, L + 2), np.float32)
    sel = np.zeros((L, KT, 128), np.float32)
    kern = np.zeros((128, KT, C), np.float32)
    ar = np.arange(128)
    for t in range(KT):
        lab_t = labels[t * 128 : (t + 1) * 128]
        oh9[ar, t, lab_t] = 1.0
        oh9[:, t, L] = 1.0
        sel[lab_t, t, ar] = 1.0
        kern[:, t, :] = kernels[t * 128 : (t + 1) * 128, :]
    return oh9, sel, kern


def _run(inputs, trace=False):
    from concourse.bass_utils import run_bass_kernel_spmd

    vmf = np.ascontiguousarray(np.asarray(inputs["vmf_activations"], dtype=np.float32))
    kernels = np.asarray(inputs["kernels"], dtype=np.float32)
    labels = np.asarray(inputs["kernel_labels"]).astype(np.int64)

    oh9, sel, kern = _make_consts(kernels, labels)

    in_maps = []
    for i in range(NCORES):
        b, h0 = i // 2, (i % 2) * 64
        shard = np.ascontiguousarray(vmf[b, :, h0 : h0 + 64, :].reshape(K, PIX))
        in_maps.append({"vmf": shard, "oh9": oh9, "sel": sel, "kern": kern})

    nc = _get_program()
    res = run_bass_kernel_spmd(nc, in_maps, core_ids=list(range(NCORES)), trace=trace)

    content = np.zeros((B, L, H, W), np.float32)
    features = np.zeros((B, C, H, W), np.float32)
    for i, rd in enumerate(res.results):
        b, h0 = i // 2, (i % 2) * 64
        features[b, :, h0 : h0 + 64, :] = rd["feat"].reshape(C, 64, W)
        ct = rd["contT"].reshape(128, NCHUNK, 4, L)
        content[b, :, h0 : h0 + 64, :] = ct.transpose(3, 1, 2, 0).reshape(L, 64, W)
    return (content, features), res


def kernel(**inputs):
    out, _ = _run(inputs, trace=False)
    return out


def _make_in_maps(inputs):
    vmf = np.ascontiguousarray(np.asarray(inputs["vmf_activations"], dtype=np.float32))
    kernels = np.asarray(inputs["kernels"], dtype=np.float32)
    labels = np.asarray(inputs["kernel_labels"]).astype(np.int64)
    oh9, sel, kern = _make_consts(kernels, labels)
    in_maps = []
    for i in range(NCORES):
        b, h0 = i // 2, (i % 2) * 64
        shard = np.ascontiguousarray(vmf[b, :, h0 : h0 + 64, :].reshape(K, PIX))
        in_maps.append({"vmf": shard, "oh9": oh9, "sel": sel, "kern": kern})
    return in_maps


def _make_timing_fn(nc, in_maps):
    """Build a non-donating jitted runner for nc; returns (fn, dev_args)."""
    import jax
    from jax.sharding import Mesh, PartitionSpec
    from jax.experimental.shard_map import shard_map
    import concourse.mybir as mybir
    from concourse import bass2jax

    bass2jax.install_neuronx_cc_hook()

    partition_name = nc.partition_id_tensor.name if nc.partition_id_tensor else None
    in_names, out_names, out_avals, zero_outs = [], [], [], []
    for alloc in nc.m.functions[0].allocations:
        if not isinstance(alloc, mybir.MemoryLocationSet):
            continue
        name = alloc.memorylocations[0].name
        if alloc.kind == "ExternalInput":
            if name != partition_name:
                in_names.append(name)
        elif alloc.kind == "ExternalOutput":
            shape = tuple(alloc.tensor_shape)
            dtype = mybir.dt.np(alloc.dtype)
            out_names.append(name)
            out_avals.append(jax.core.ShapedArray(shape, dtype))
            zero_outs.append(np.zeros(shape, dtype))
    n_params = len(in_names)
    all_in_names = in_names + out_names
    if partition_name is not None:
        all_in_names = all_in_names + [partition_name]

    def _body(*args):
        operands = list(args)
        if partition_name is not None:
            operands.append(bass2jax.partition_id_tensor())
        outs = bass2jax._bass_exec_p.bind(
            *operands,
            out_avals=tuple(out_avals),
            in_names=tuple(all_in_names),
            out_names=tuple(out_names),
            lowering_input_output_aliases=(),
            sim_require_finite=True,
            sim_require_nnan=True,
            nc=nc,
        )
        return tuple(outs)

    devices = jax.devices()[:NCORES]
    mesh = Mesh(np.asarray(devices), ("core",))
    n_outs = len(out_names)
    in_specs = (PartitionSpec("core"),) * (n_params + n_outs)
    out_specs = (PartitionSpec("core"),) * n_outs
    fn = jax.jit(
        shard_map(_body, mesh=mesh, in_specs=in_specs, out_specs=out_specs,
                  check_rep=False),
        keep_unused=True,
    )
    concat_in = [
        np.concatenate([np.asarray(m[nm]) for m in in_maps], axis=0)
        for nm in in_names
    ]
    concat_zero = [
        np.zeros((NCORES * z.shape[0], *z.shape[1:]), z.dtype) for z in zero_outs
    ]
    args = [jax.device_put(a) for a in concat_in + concat_zero]
    return fn, args


def _make_timing_fn(nc, in_maps):
    """Build a non-donating jitted runner for nc; returns (fn, dev_args)."""
    import jax
    from jax.sharding import Mesh, PartitionSpec
    from jax.experimental.shard_map import shard_map
    import concourse.mybir as mybir
    from concourse import bass2jax

    bass2jax.install_neuronx_cc_hook()

    partition_name = nc.partition_id_tensor.name if nc.partition_id_tensor else None
    in_names, out_names, out_avals, zero_outs = [], [], [], []
    for alloc in nc.m.functions[0].allocations:
        if not isinstance(alloc, mybir.MemoryLocationSet):
            continue
        name = alloc.memorylocations[0].name
        if alloc.kind == "ExternalInput":
            if name != partition_name:
                in_names.append(name)
        elif alloc.kind == "ExternalOutput":
            shape = tuple(alloc.tensor_shape)
            dtype = mybir.dt.np(alloc.dtype)
            out_names.append(name)
            out_avals.append(jax.core.ShapedArray(shape, dtype))
            zero_outs.append(np.zeros(shape, dtype))
    n_params = len(in_names)
    all_in_names = in_names + out_names
    if partition_name is not None:
        all_in_names = all_in_names + [partition_name]

    def _body(*args):
        operands = list(args)
        if partition_name is not None:
            operands.append(bass2jax.partition_id_tensor())
        outs = bass2jax._bass_exec_p.bind(
            *operands,
            out_avals=tuple(out_avals),
            in_names=tuple(all_in_names),
            out_names=tuple(out_names),
            lowering_input_output_aliases=(),
            sim_require_finite=True,
            sim_require_nnan=True,
            nc=nc,
        )
        return tuple(outs)

    devices = jax.devices()[:NCORES]
    mesh = Mesh(np.asarray(devices), ("core",))
    n_outs = len(out_names)
    in_specs = (PartitionSpec("core"),) * (n_params + n_outs)
    out_specs = (PartitionSpec("core"),) * n_outs
    fn = jax.jit(
        shard_map(_body, mesh=mesh, in_specs=in_specs, out_specs=out_specs,
                  check_rep=False),
        keep_unused=True,
    )
    concat_in = [
        np.concatenate([np.asarray(m[nm]) for m in in_maps], axis=0)
        for nm in in_names
    ]
    concat_zero = [
        np.zeros((NCORES * z.shape[0], *z.shape[1:]), z.dtype) for z in zero_outs
    ]
    args = [jax.device_put(a) for a in concat_in + concat_zero]
    return fn, args


def time_hw(inputs, iters=40, repn=5):
    """Device time per workload pass, measured as the marginal wall-clock cost
    of extra on-device repetitions: (T(repn) - T(1)) / (repn - 1).  Calls of
    the two variants are interleaved and differenced pairwise (adjacent calls
    see the same axon round-trip conditions), then the median difference is
    taken — per-dispatch overhead cancels, slow RTT drift cancels too."""
    import jax
    import time as _time

    in_maps = _make_in_maps(inputs)
    fn1, args1 = _make_timing_fn(_get_program(rep=1), in_maps)
    fnN, argsN = _make_timing_fn(_get_program(rep=repn), in_maps)

    def once(fn, args):
        t0 = _time.perf_counter()
        jax.block_until_ready(fn(*args))
        return (_time.perf_counter() - t0) * 1e9

    for _ in range(4):  # warm both paths
        once(fn1, args1), once(fnN, argsN)
    diffs = []
    t1s, tNs = [], []
    for _ in range(iters):
        t1 = once(fn1, args1)
        tN = once(fnN, argsN)
        t1s.append(t1)
        tNs.append(tN)
        diffs.append(tN - t1)
    med = float(np.median(diffs))
    print(
        f"  [time_hw] med T1={np.median(t1s):.0f} med T{repn}={np.median(tNs):.0f} "
        f"med diff={med:.0f} (p25={np.percentile(diffs,25):.0f}, "
        f"p75={np.percentile(diffs,75):.0f}) ns"
    )
    return med / (repn - 1)


def time_hw_ntff(inputs, n=3):
    """True HW exec time via NTFF profiling (axon hook shim)."""
    import ntff_hook

    ntff_hook.install()
    import os

    os.environ["BASS_PERFETTO_PROFILE_ALL_CORES"] = "1"
    best = None
    for _ in range(n):
        _, res = _run(inputs, trace=True)
        if res.exec_time_ns is not None:
            best = res.exec_time_ns if best is None else min(best, res.exec_time_ns)
    return best
